# revision 1
# baseline (speedup 1.0000x reference)
"""GCN encoder (2x GCNConv + MLP proj head) on 8 Trainium2 NeuronCores.

Strategy: shard nodes across the 8 cores (1250/core, padded to 1280).
The symmetric GCN norm dis[src]*dis[dst] factors into per-node pre/post
scaling, so each aggregation round is: per-core dense matmul (X@W, bf16,
f32 PSUM) + dis-scale -> AllGather of the scaled features -> per 128-dst
window: dma_gather of deduped source rows (split across SWDGE queues) +
host-precomputed one-hot/count scatter matmuls accumulating segment sums
in PSUM (self-loop term folded in via an identity matmul against the
locally resident scaled features) -> dis post-scale on the scalar engine.
The proj head is purely local matmuls overlapped with the second
AllGather.
"""
import json

import numpy as np
import ml_dtypes

N = 10000
E = 160000
D = 512
NC = 8
NPC = N // NC  # 1250 nodes per core
CH = 10  # 128-node chunks / windows per core
NPAD = CH * 128  # 1280

_BF16 = ml_dtypes.bfloat16

_WAIT_SPLIT_DONE = False


def _install_wait_split():
    """This container's walrus rejects instructions with >1 sync wait.
    Hoist extra waits onto single-wait Drain instructions just before the
    instruction on the same engine (same sequencer => same semantics)."""
    global _WAIT_SPLIT_DONE
    if _WAIT_SPLIT_DONE:
        return
    _WAIT_SPLIT_DONE = True
    import concourse.bass as bass

    orig = bass.Bass.to_json_bytes

    def _split_block(instructions):
        out = []
        changed = False
        for inst in instructions:
            sync = inst.get("sync_info")
            waits = (sync or {}).get("on_wait") or []
            if len(waits) > 1:
                changed = True
                for j, w in enumerate(waits[:-1]):
                    out.append(
                        {
                            "engine": inst["engine"],
                            "ins": [],
                            "name": f"{inst['name']}-wsplit{j}",
                            "opcode": "Drain",
                            "outs": [],
                            "sync_info": {"on_update": [], "on_wait": [w]},
                        }
                    )
                sync["on_wait"] = waits[-1:]
            out.append(inst)
        return out, changed

    def to_json_bytes(self):
        js = json.loads(orig(self))
        stack = [js]
        while stack:
            d = stack.pop()
            if isinstance(d, dict):
                if "instructions" in d:
                    new, changed = _split_block(d["instructions"])
                    if changed:
                        d["instructions"] = new
                for v in d.values():
                    if isinstance(v, (dict, list)):
                        stack.append(v)
            elif isinstance(d, list):
                stack.extend(d)
        return json.dumps(js).encode()

    bass.Bass.to_json_bytes = to_json_bytes


def _split3(k_cw):
    """Split k_cw chunks into 2-chunk gather pieces (8-way split across the
    4 SWDGE queues measured fastest for desc-gen concurrency)."""
    out = [2] * (k_cw // 2)
    if k_cw % 2:
        out.append(1)
    return out


def _build_program(k_cw, has_b1, has_b2, has_bp1, has_bp2):
    import concourse.bass as bass
    import concourse.tile as tile
    from concourse import mybir
    from concourse.library_config import mlp
    from concourse.library_overlay import lower_extended_insts
    from concourse.tile_rust import add_dep_helper

    f32 = mybir.dt.float32
    bf16 = mybir.dt.bfloat16
    i16 = mybir.dt.int16
    ACTF = mybir.ActivationFunctionType

    nc = bass.Bass(num_swdge_queues=4)

    # ---- external inputs (per-core layouts prepared on host) ----
    xt_ext = nc.dram_tensor("xt", [128, 4 * NPAD], bf16, kind="ExternalInput")
    w_ext = {
        nm: nc.dram_tensor(nm, [128, 4 * D], bf16, kind="ExternalInput")
        for nm in ("w1t", "w2t", "wp1t", "wp2t")
    }
    dis_ext = nc.dram_tensor("dis", [128, CH], f32, kind="ExternalInput")
    idx_ext = nc.dram_tensor(
        "idx16", [128, CH * k_cw * 8], i16, kind="ExternalInput"
    )
    s_ext = nc.dram_tensor(
        "stab", [128, CH * k_cw * 128], bf16, kind="ExternalInput"
    )
    ident_ext = nc.dram_tensor("ident", [128, 128], bf16, kind="ExternalInput")
    b_ext = {}
    for nm, has in (
        ("b1", has_b1),
        ("b2", has_b2),
        ("bp1", has_bp1),
        ("bp2", has_bp2),
    ):
        if has:
            b_ext[nm] = nc.dram_tensor(nm, [128, D], f32, kind="ExternalInput")

    # ---- external outputs ----
    z_out = nc.dram_tensor("z", [NPAD, D], f32, kind="ExternalOutput")
    out_out = nc.dram_tensor("agg", [NPAD, D], f32, kind="ExternalOutput")
    proj_out = nc.dram_tensor("proj", [NPAD, D], f32, kind="ExternalOutput")

    # ---- internal DRAM ----
    HALF = NPAD // 2  # 640
    h1p_sh = nc.dram_tensor("h1p_sh", [NPAD, D], bf16)
    h1p_full = nc.dram_tensor("h1p_full", [NC * NPAD, D], bf16, addr_space="Shared")
    h2p_sh = nc.dram_tensor("h2p_sh", [NPAD, D], bf16)
    h2p_full = nc.dram_tensor("h2p_full", [NC * NPAD, D], bf16, addr_space="Shared")

    core_ids = list(range(NC))
    splits = _split3(k_cw)

    with tile.TileContext(nc) as tc:
        with (
            tc.tile_pool(name="const", bufs=1) as cpool,
            tc.tile_pool(name="work", bufs=3) as wpool,
            tc.tile_pool(name="gat", bufs=3) as gpool,
            tc.tile_pool(name="tp", bufs=1) as tpool,
            tc.tile_pool(name="psA", bufs=2, space="PSUM") as psA,
            tc.tile_pool(name="psB", bufs=2, space="PSUM") as psB,
        ):
            lib_inst = nc.gpsimd.load_library(mlp)
            # one shared register per distinct gather size (to_reg per call
            # would exhaust the Pool register file at 60 gathers)
            nidx_regs = {
                nk: nc.gpsimd.to_reg(nk * 128) for nk in sorted(set(splits))
            }

            # ---- phase-critical constant loads (sync/SP HWDGE ring) ----
            xt_t = cpool.tile([128, 4 * NPAD], bf16)
            nc.sync.dma_start(xt_t[:], xt_ext[:])
            w_t = {}
            for nm in ("w1t", "w2t", "wp1t", "wp2t"):
                w_t[nm] = cpool.tile([128, 4 * D], bf16, tag=nm, name=nm)
                nc.sync.dma_start(w_t[nm][:], w_ext[nm][:])
            dis_t = cpool.tile([128, CH], f32)
            nc.sync.dma_start(dis_t[:], dis_ext[:])
            ident_t = cpool.tile([128, 128], bf16)
            nc.sync.dma_start(ident_t[:], ident_ext[:])
            b_t = {}
            for nm in b_ext:
                b_t[nm] = cpool.tile([128, D], f32, tag=nm, name=nm + "_bc")
                nc.sync.dma_start(b_t[nm][:], b_ext[nm][:])
            # ---- bulk loads on the scalar HWDGE ring (parallel to above) ----
            idx_t = cpool.tile([128, CH * k_cw * 8], i16)
            nc.scalar.dma_start(idx_t[:], idx_ext[:])
            s_t = cpool.tile([128, CH * k_cw * 128], bf16)
            nc.scalar.dma_start(s_t[:], s_ext[:])

            # persistent scaled-feature chunks (self-loop term source)
            h1p_t = cpool.tile([128, CH * D], bf16)
            h2p_t = cpool.tile([128, CH * D], bf16)

            def dense_layer(lhs_tiles, w_name, m):
                ps = psA.tile([128, D], f32, tag="dense")
                for kk in range(4):
                    nc.tensor.matmul(
                        ps[:],
                        lhs_tiles(kk, m),
                        w_t[w_name][:, kk * D : (kk + 1) * D],
                        start=(kk == 0),
                        stop=(kk == 3),
                    )
                return ps

            def xt_tile(kk, m):
                return xt_t[:, kk * NPAD + m * 128 : kk * NPAD + (m + 1) * 128]

            def scale_to(dst_ap, ps, m, bias_name):
                """dst = dis_m * (ps + bias) via ACT (bias pre-add on DVE)."""
                if bias_name in b_t:
                    tmp = wpool.tile([128, D], f32, tag="btmp")
                    nc.vector.tensor_tensor(
                        tmp[:], ps[:], b_t[bias_name][:], op=mybir.AluOpType.add
                    )
                    src = tmp
                else:
                    src = ps
                nc.scalar.activation(
                    dst_ap, src[:], ACTF.Copy, scale=dis_t[:, m : m + 1]
                )

            # ---- phase A: H1' = dis * (X @ W1 + b1), own nodes ----
            for m in range(CH):
                ps = dense_layer(xt_tile, "w1t", m)
                scale_to(h1p_t[:, m * D : (m + 1) * D], ps, m, "b1")
                nc.sync.dma_start(
                    h1p_sh[m * 128 : (m + 1) * 128, :],
                    h1p_t[:, m * D : (m + 1) * D],
                )
                if m == 4:
                    nc.gpsimd.collective_compute(
                        "AllGather",
                        mybir.AluOpType.bypass,
                        ins=[h1p_sh[0:HALF, :]],
                        outs=[h1p_full[0 : NC * HALF, :]],
                        replica_groups=[core_ids],
                    )
            nc.gpsimd.collective_compute(
                "AllGather",
                mybir.AluOpType.bypass,
                ins=[h1p_sh[HALF:NPAD, :]],
                outs=[h1p_full[NC * HALF : 2 * NC * HALF, :]],
                replica_groups=[core_ids],
            )

            def agg_round(src_full, selfsrc_t):
                for w in range(CH):
                    g_t = gpool.tile([128, k_cw, D], bf16, tag="g", name=f"g{w}")
                    a = 0
                    for hh, nk in enumerate(splits):
                        gi = nc.gpsimd.dma_gather(
                            g_t[:, a : a + nk, :],
                            src_full[:],
                            idx_t[
                                :,
                                (w * k_cw + a) * 8 : (w * k_cw + a + nk) * 8,
                            ],
                            num_idxs=nk * 128,
                            num_idxs_reg=nidx_regs[nk],
                            elem_size=D,
                            single_packet=False,
                            queue_num=hh % 4,
                        )
                        add_dep_helper(
                            getattr(gi, "ins", gi),
                            getattr(lib_inst, "ins", lib_inst),
                            reason="mlp library before dma_gather",
                        )
                        a += nk
                    ps = psB.tile([128, D], f32, tag="agg", name=f"agg{w}")
                    for k in range(k_cw):
                        sc = (w * k_cw + k) * 128
                        nc.tensor.matmul(
                            ps[:],
                            s_t[:, sc : sc + 128],
                            g_t[:, k, :],
                            start=(k == 0),
                            stop=False,
                        )
                    # self-loop term: ps += I.T @ h'_w
                    nc.tensor.matmul(
                        ps[:],
                        ident_t[:],
                        selfsrc_t[:, w * D : (w + 1) * D],
                        start=False,
                        stop=True,
                    )
                    yield w, ps

            # ---- fused round 1 + L2 + proj head, pipelined per window ----
            zt_t = tpool.tile([128, 4 * NPAD], bf16, tag="zt")
            rt_t = tpool.tile([128, 4 * NPAD], bf16, tag="rt")
            p1t_t = tpool.tile([128, 4 * NPAD], bf16, tag="p1t")

            def zt_tile(kk, m):
                return zt_t[:, kk * NPAD + m * 128 : kk * NPAD + (m + 1) * 128]

            def rt_tile(kk, m):
                return rt_t[:, kk * NPAD + m * 128 : kk * NPAD + (m + 1) * 128]

            def p1t_tile(kk, m):
                return p1t_t[:, kk * NPAD + m * 128 : kk * NPAD + (m + 1) * 128]

            for w, ps in agg_round(h1p_full, h1p_t):
                z_f = wpool.tile([128, D], f32, tag="zf")
                nc.scalar.activation(
                    z_f[:], ps[:], ACTF.Copy, scale=dis_t[:, w : w + 1]
                )
                z_b = wpool.tile([128, D], bf16, tag="zb")
                nc.scalar.activation(
                    z_b[:], ps[:], ACTF.Copy, scale=dis_t[:, w : w + 1]
                )
                nc.sync.dma_start(z_out[w * 128 : (w + 1) * 128, :], z_f[:])
                # transpose z chunk into zt columns; relu'd copy into rt
                psT = psA.tile([128, 4, 128], bf16, tag="tr", name=f"tr{w}")
                for kk in range(4):
                    nc.tensor.transpose(
                        psT[:, kk, :],
                        z_b[:, kk * 128 : (kk + 1) * 128],
                        ident_t[:],
                    )
                zt_cols = zt_t[:].rearrange("p (k n) -> p k n", n=NPAD)[
                    :, :, w * 128 : (w + 1) * 128
                ]
                rt_cols = rt_t[:].rearrange("p (k n) -> p k n", n=NPAD)[
                    :, :, w * 128 : (w + 1) * 128
                ]
                nc.vector.tensor_copy(zt_cols, psT[:])
                nc.vector.tensor_scalar(
                    rt_cols, psT[:], 0.0, None, op0=mybir.AluOpType.max
                )
                # L2 for this node chunk -> H2' shard
                ps2 = dense_layer(rt_tile, "w2t", w)
                scale_to(h2p_t[:, w * D : (w + 1) * D], ps2, w, "b2")
                nc.sync.dma_start(
                    h2p_sh[w * 128 : (w + 1) * 128, :],
                    h2p_t[:, w * D : (w + 1) * D],
                )
                # proj first layer for this chunk
                ps3 = dense_layer(zt_tile, "wp1t", w)
                p1_b = wpool.tile([128, D], bf16, tag="p1")
                if "bp1" in b_t:
                    tmp = wpool.tile([128, D], f32, tag="btmp")
                    nc.vector.tensor_tensor(
                        tmp[:], ps3[:], b_t["bp1"][:], op=mybir.AluOpType.add
                    )
                    nc.vector.tensor_scalar(
                        p1_b[:], tmp[:], 0.0, None, op0=mybir.AluOpType.max
                    )
                else:
                    nc.vector.tensor_scalar(
                        p1_b[:], ps3[:], 0.0, None, op0=mybir.AluOpType.max
                    )
                psT2 = psA.tile([128, 4, 128], bf16, tag="tr", name=f"tr2{w}")
                for kk in range(4):
                    nc.tensor.transpose(
                        psT2[:, kk, :],
                        p1_b[:, kk * 128 : (kk + 1) * 128],
                        ident_t[:],
                    )
                p1t_cols = p1t_t[:].rearrange("p (k n) -> p k n", n=NPAD)[
                    :, :, w * 128 : (w + 1) * 128
                ]
                nc.vector.tensor_copy(p1t_cols, psT2[:])
                # proj second layer for this chunk
                ps4 = dense_layer(p1t_tile, "wp2t", w)
                pj_t = wpool.tile([128, D], f32, tag="pj")
                if "bp2" in b_t:
                    nc.vector.tensor_tensor(
                        pj_t[:], ps4[:], b_t["bp2"][:], op=mybir.AluOpType.add
                    )
                else:
                    nc.vector.tensor_copy(pj_t[:], ps4[:])
                nc.sync.dma_start(proj_out[w * 128 : (w + 1) * 128, :], pj_t[:])
                if w == 4:
                    nc.gpsimd.collective_compute(
                        "AllGather",
                        mybir.AluOpType.bypass,
                        ins=[h2p_sh[0:HALF, :]],
                        outs=[h2p_full[0 : NC * HALF, :]],
                        replica_groups=[core_ids],
                    )
            nc.gpsimd.collective_compute(
                "AllGather",
                mybir.AluOpType.bypass,
                ins=[h2p_sh[HALF:NPAD, :]],
                outs=[h2p_full[NC * HALF : 2 * NC * HALF, :]],
                replica_groups=[core_ids],
            )

            # ---- phase E: round 2 -> out ----
            for w, ps in agg_round(h2p_full, h2p_t):
                o_f = wpool.tile([128, D], f32, tag="of")
                nc.scalar.activation(
                    o_f[:], ps[:], ACTF.Copy, scale=dis_t[:, w : w + 1]
                )
                nc.sync.dma_start(out_out[w * 128 : (w + 1) * 128, :], o_f[:])

    lower_extended_insts(nc)
    return nc


def _host_prep(x, edge_index, W1, W2, Wp1, Wp2):
    src = np.asarray(edge_index[0], np.int64)
    dst = np.asarray(edge_index[1], np.int64)

    # degree includes self loops (norm definition), but self edges are
    # handled on-device via the identity matmul, not the gather.
    deg = (np.bincount(np.concatenate([dst, np.arange(N)]), minlength=N)).astype(
        np.float32
    )
    dis = (1.0 / np.sqrt(np.maximum(deg, 1.0))).astype(np.float32)

    owner = src // NPC
    local = src - owner * NPC
    HALF = NPAD // 2
    # AllGather halves land rank-major per half: [8*640 | 8*640]
    gather_row = np.where(
        local < HALF,
        owner * HALF + local,
        NC * HALF + owner * HALF + (local - HALF),
    )

    dst_core = dst // NPC
    dst_local = dst - dst_core * NPC  # [0, 1250)
    win = dst_local // 128
    dloc = dst_local - win * 128

    order = np.lexsort((dst_local, dst_core))
    g_sorted = gather_row[order]
    dc = dst_core[order]
    wn = win[order]
    dl = dloc[order]

    counts = np.zeros((NC, CH), np.int64)
    np.add.at(counts, (dc, wn), 1)
    flat_counts = counts.reshape(-1)
    starts = np.concatenate([[0], np.cumsum(flat_counts)])[:-1].reshape(NC, CH)

    # dedup per (core, window); k_cw from max unique count
    uniq = {}
    max_u = 0
    for c in range(NC):
        for w in range(CH):
            s0, n = starts[c, w], counts[c, w]
            rows = g_sorted[s0 : s0 + n]
            dd = dl[s0 : s0 + n]
            u, inv = np.unique(rows, return_inverse=True)
            uniq[(c, w)] = (u, inv, dd)
            max_u = max(max_u, len(u))
    k_cw = int(np.ceil(max_u / 128))
    wlen = k_cw * 128

    per_core = []
    for c in range(NC):
        idx_pad = np.zeros((CH, wlen), np.int64)
        s_tab = np.zeros((CH, wlen, 128), np.float32)
        for w in range(CH):
            u, inv, dd = uniq[(c, w)]
            idx_pad[w, : len(u)] = u
            np.add.at(s_tab[w], (inv, dd), 1.0)

        iw = idx_pad.reshape(CH, wlen // 16, 16).transpose(0, 2, 1)
        idx16 = np.tile(iw, (1, 8, 1)).transpose(1, 0, 2).reshape(128, -1)
        idx16 = np.ascontiguousarray(idx16, np.int16)

        # stab: [128, CH*k_cw*128]; col (w*k_cw+k)*128+d, part p = S[w, k*128+p, d]
        stab = (
            s_tab.reshape(CH, k_cw, 128, 128)
            .transpose(2, 0, 1, 3)
            .reshape(128, -1)
        )
        stab = np.ascontiguousarray(stab).astype(_BF16)

        xc = np.zeros((NPAD, D), np.float32)
        xc[:NPC] = x[c * NPC : (c + 1) * NPC]
        xt = xc.T.reshape(4, 128, NPAD).transpose(1, 0, 2).reshape(128, -1)
        xt = np.ascontiguousarray(xt).astype(_BF16)

        dis_c = np.zeros((NPAD,), np.float32)
        dis_c[:NPC] = dis[c * NPC : (c + 1) * NPC]
        dis_t = np.ascontiguousarray(dis_c.reshape(CH, 128).T, np.float32)

        per_core.append(
            {"xt": xt, "idx16": idx16, "stab": stab, "dis": dis_t}
        )

    def wtile(W):
        wt = (
            np.asarray(W, np.float32)
            .reshape(4, 128, D)
            .transpose(1, 0, 2)
            .reshape(128, -1)
        )
        return np.ascontiguousarray(wt).astype(_BF16)

    shared = {
        "w1t": wtile(W1),
        "w2t": wtile(W2),
        "wp1t": wtile(Wp1),
        "wp2t": wtile(Wp2),
        "ident": np.eye(128, dtype=np.float32).astype(_BF16),
    }
    return k_cw, per_core, shared


def run(inputs, trace=False, **run_kwargs):
    """Build + run; returns ((out, z, proj), BassKernelResults)."""
    _install_wait_split()
    from concourse.bass_utils import run_bass_kernel_spmd

    x = np.asarray(inputs["x"], np.float32)
    b1, b2 = inputs["b1"], inputs["b2"]
    bp1, bp2 = inputs["bp1"], inputs["bp2"]
    k_cw, per_core, shared = _host_prep(
        x, inputs["edge_index"], inputs["W1"], inputs["W2"], inputs["Wp1"],
        inputs["Wp2"],
    )

    has_b = {
        "b1": bool(np.any(np.asarray(b1))),
        "b2": bool(np.any(np.asarray(b2))),
        "bp1": bool(np.any(np.asarray(bp1))),
        "bp2": bool(np.any(np.asarray(bp2))),
    }
    nc = _build_program(
        k_cw, has_b["b1"], has_b["b2"], has_b["bp1"], has_b["bp2"]
    )

    in_maps = []
    for c in range(NC):
        m = dict(per_core[c])
        m.update(shared)
        for nm, b in (("b1", b1), ("b2", b2), ("bp1", bp1), ("bp2", bp2)):
            if has_b[nm]:
                m[nm] = np.ascontiguousarray(
                    np.tile(np.asarray(b, np.float32)[None, :], (128, 1))
                )
        in_maps.append(m)

    res = run_bass_kernel_spmd(
        nc, in_maps, core_ids=list(range(NC)), trace=trace, **run_kwargs
    )

    out = np.empty((N, D), np.float32)
    z = np.empty((N, D), np.float32)
    proj = np.empty((N, D), np.float32)
    for c in range(NC):
        r = res.results[c]
        out[c * NPC : (c + 1) * NPC] = r["agg"][:NPC]
        z[c * NPC : (c + 1) * NPC] = r["z"][:NPC]
        proj[c * NPC : (c + 1) * NPC] = r["proj"][:NPC]
    return (out, z, proj), res


def kernel(x, edge_index, W1, b1, W2, b2, Wp1, bp1, Wp2, bp2):
    outs, _ = run(
        {
            "x": x, "edge_index": edge_index, "W1": W1, "b1": b1,
            "W2": W2, "b2": b2, "Wp1": Wp1, "bp1": bp1,
            "Wp2": Wp2, "bp2": bp2,
        }
    )
    return outs



# revision 5
# speedup vs baseline: 1.4757x; 1.4757x over previous
"""GCN encoder (2x GCNConv + MLP proj head) on 8 Trainium2 NeuronCores.

Strategy: shard nodes across the 8 cores (1250/core, padded to 1280).
The symmetric GCN norm dis[src]*dis[dst] factors into per-node pre/post
scaling, so each aggregation round is: per-core dense matmul (X@W, bf16,
f32 PSUM) + dis-scale -> AllGather of the scaled features -> per 128-dst
window: dma_gather of deduped source rows (split across SWDGE queues) +
host-precomputed one-hot/count scatter matmuls accumulating segment sums
in PSUM (self-loop term folded in via an identity matmul against the
locally resident scaled features) -> dis post-scale on the scalar engine.
The proj head is purely local matmuls overlapped with the second
AllGather.
"""
import json

import numpy as np
import ml_dtypes

N = 10000
E = 160000
D = 512
NC = 8
NPC = N // NC  # 1250 nodes per core
CH = 10  # 128-node chunks / windows per core
NPAD = CH * 128  # 1280

_BF16 = ml_dtypes.bfloat16

_WAIT_SPLIT_DONE = False


def _install_wait_split():
    """This container's walrus rejects instructions with >1 sync wait.
    Hoist extra waits onto single-wait Drain instructions just before the
    instruction on the same engine (same sequencer => same semantics)."""
    global _WAIT_SPLIT_DONE
    if _WAIT_SPLIT_DONE:
        return
    _WAIT_SPLIT_DONE = True
    import concourse.bass as bass

    orig = bass.Bass.to_json_bytes

    def _split_block(instructions):
        out = []
        changed = False
        for inst in instructions:
            sync = inst.get("sync_info")
            waits = (sync or {}).get("on_wait") or []
            if len(waits) > 1:
                changed = True
                for j, w in enumerate(waits[:-1]):
                    out.append(
                        {
                            "engine": inst["engine"],
                            "ins": [],
                            "name": f"{inst['name']}-wsplit{j}",
                            "opcode": "Drain",
                            "outs": [],
                            "sync_info": {"on_update": [], "on_wait": [w]},
                        }
                    )
                sync["on_wait"] = waits[-1:]
            out.append(inst)
        return out, changed

    def to_json_bytes(self):
        js = json.loads(orig(self))
        stack = [js]
        while stack:
            d = stack.pop()
            if isinstance(d, dict):
                if "instructions" in d:
                    new, changed = _split_block(d["instructions"])
                    if changed:
                        d["instructions"] = new
                for v in d.values():
                    if isinstance(v, (dict, list)):
                        stack.append(v)
            elif isinstance(d, list):
                stack.extend(d)
        return json.dumps(js).encode()

    bass.Bass.to_json_bytes = to_json_bytes


def _split3(k_cw):
    """Split k_cw chunks into 2-chunk gather pieces (8-way split across the
    4 SWDGE queues measured fastest for desc-gen concurrency)."""
    out = [2] * (k_cw // 2)
    if k_cw % 2:
        out.append(1)
    return out


def _build_program(k_cw, has_b1, has_b2, has_bp1, has_bp2):
    import concourse.bass as bass
    import concourse.tile as tile
    from concourse import mybir
    from concourse.library_config import mlp
    from concourse.library_overlay import lower_extended_insts
    from concourse.tile_rust import add_dep_helper

    f32 = mybir.dt.float32
    bf16 = mybir.dt.bfloat16
    i16 = mybir.dt.int16
    ACTF = mybir.ActivationFunctionType

    # 3x default SWDGE descriptor scratch (12KB/queue): a 2-chunk gather
    # (256 descriptors, 4KB) no longer fills its queue's ring, so the
    # fused desc-gen returns without blocking for the in-flight DMA.
    nc = bass.Bass(num_swdge_queues=4, dynamic_dma_scratch_size=49152)

    # ---- external inputs (per-core layouts prepared on host) ----
    xt_ext = nc.dram_tensor("xt", [128, 4 * NPAD], bf16, kind="ExternalInput")
    w_ext = {
        nm: nc.dram_tensor(nm, [128, 4 * D], bf16, kind="ExternalInput")
        for nm in ("w1t", "w2t", "wp1t", "wp2t")
    }
    dis_ext = nc.dram_tensor("dis", [128, CH], f32, kind="ExternalInput")
    idx_ext = nc.dram_tensor(
        "idx16", [128, CH * k_cw * 8], i16, kind="ExternalInput"
    )
    s_ext = nc.dram_tensor(
        "stab", [128, CH * k_cw * 128], bf16, kind="ExternalInput"
    )
    ident_ext = nc.dram_tensor("ident", [128, 128], bf16, kind="ExternalInput")
    b_ext = {}
    for nm, has in (
        ("b1", has_b1),
        ("b2", has_b2),
        ("bp1", has_bp1),
        ("bp2", has_bp2),
    ):
        if has:
            b_ext[nm] = nc.dram_tensor(nm, [128, D], f32, kind="ExternalInput")

    # ---- external outputs ----
    z_out = nc.dram_tensor("z", [NPAD, D], f32, kind="ExternalOutput")
    out_out = nc.dram_tensor("agg", [NPAD, D], f32, kind="ExternalOutput")
    proj_out = nc.dram_tensor("proj", [NPAD, D], f32, kind="ExternalOutput")

    # ---- internal DRAM ----
    HALF = NPAD // 2  # 640
    h1p_sh = nc.dram_tensor("h1p_sh", [NPAD, D], bf16)
    h1p_full = nc.dram_tensor("h1p_full", [NC * NPAD, D], bf16, addr_space="Shared")
    h2p_sh = nc.dram_tensor("h2p_sh", [NPAD, D], bf16)
    h2p_full = nc.dram_tensor("h2p_full", [NC * NPAD, D], bf16, addr_space="Shared")

    core_ids = list(range(NC))
    splits = _split3(k_cw)

    with tile.TileContext(nc) as tc:
        with (
            tc.tile_pool(name="const", bufs=1) as cpool,
            tc.tile_pool(name="work", bufs=3) as wpool,
            tc.tile_pool(name="gat", bufs=3) as gpool,
            tc.tile_pool(name="tp", bufs=1) as tpool,
            tc.tile_pool(name="psA", bufs=2, space="PSUM") as psA,
            tc.tile_pool(name="psB", bufs=2, space="PSUM") as psB,
        ):
            lib_inst = nc.gpsimd.load_library(mlp)
            # one shared register per distinct gather size (to_reg per call
            # would exhaust the Pool register file at 60 gathers)
            nidx_regs = {
                nk: nc.gpsimd.to_reg(nk * 128) for nk in sorted(set(splits))
            }

            # ---- phase-critical constant loads (sync/SP HWDGE ring) ----
            xt_t = cpool.tile([128, 4 * NPAD], bf16)
            nc.sync.dma_start(xt_t[:], xt_ext[:])
            w_t = {}
            for nm in ("w1t", "w2t", "wp1t", "wp2t"):
                w_t[nm] = cpool.tile([128, 4 * D], bf16, tag=nm, name=nm)
                nc.sync.dma_start(w_t[nm][:], w_ext[nm][:])
            dis_t = cpool.tile([128, CH], f32)
            nc.sync.dma_start(dis_t[:], dis_ext[:])
            ident_t = cpool.tile([128, 128], bf16)
            nc.sync.dma_start(ident_t[:], ident_ext[:])
            b_t = {}
            for nm in b_ext:
                b_t[nm] = cpool.tile([128, D], f32, tag=nm, name=nm + "_bc")
                nc.sync.dma_start(b_t[nm][:], b_ext[nm][:])
            # ---- bulk loads on the scalar HWDGE ring (parallel to above) ----
            idx_t = cpool.tile([128, CH * k_cw * 8], i16)
            nc.scalar.dma_start(idx_t[:], idx_ext[:])
            s_t = cpool.tile([128, CH * k_cw * 128], bf16)
            nc.scalar.dma_start(s_t[:], s_ext[:])

            # persistent scaled-feature chunks (self-loop term source)
            h1p_t = cpool.tile([128, CH * D], bf16)
            h2p_t = cpool.tile([128, CH * D], bf16)

            def dense_layer(lhs_tiles, w_name, m):
                ps = psA.tile([128, D], f32, tag="dense")
                for kk in range(4):
                    nc.tensor.matmul(
                        ps[:],
                        lhs_tiles(kk, m),
                        w_t[w_name][:, kk * D : (kk + 1) * D],
                        start=(kk == 0),
                        stop=(kk == 3),
                    )
                return ps

            def xt_tile(kk, m):
                return xt_t[:, kk * NPAD + m * 128 : kk * NPAD + (m + 1) * 128]

            def scale_to(dst_ap, ps, m, bias_name):
                """dst = dis_m * (ps + bias) via ACT (bias pre-add on DVE)."""
                if bias_name in b_t:
                    tmp = wpool.tile([128, D], f32, tag="btmp")
                    nc.vector.tensor_tensor(
                        tmp[:], ps[:], b_t[bias_name][:], op=mybir.AluOpType.add
                    )
                    src = tmp
                else:
                    src = ps
                nc.scalar.activation(
                    dst_ap, src[:], ACTF.Copy, scale=dis_t[:, m : m + 1]
                )

            # ---- phase A: H1' = dis * (X @ W1 + b1), own nodes ----
            for m in range(CH):
                ps = dense_layer(xt_tile, "w1t", m)
                scale_to(h1p_t[:, m * D : (m + 1) * D], ps, m, "b1")
                nc.sync.dma_start(
                    h1p_sh[m * 128 : (m + 1) * 128, :],
                    h1p_t[:, m * D : (m + 1) * D],
                )
                if m == 4:
                    nc.gpsimd.collective_compute(
                        "AllGather",
                        mybir.AluOpType.bypass,
                        ins=[h1p_sh[0:HALF, :]],
                        outs=[h1p_full[0 : NC * HALF, :]],
                        replica_groups=[core_ids],
                    )
            nc.gpsimd.collective_compute(
                "AllGather",
                mybir.AluOpType.bypass,
                ins=[h1p_sh[HALF:NPAD, :]],
                outs=[h1p_full[NC * HALF : 2 * NC * HALF, :]],
                replica_groups=[core_ids],
            )

            def agg_round(src_full, selfsrc_t):
                for w in range(CH):
                    g_t = gpool.tile([128, k_cw, D], bf16, tag="g", name=f"g{w}")
                    a = 0
                    for hh, nk in enumerate(splits):
                        gi = nc.gpsimd.dma_gather(
                            g_t[:, a : a + nk, :],
                            src_full[:],
                            idx_t[
                                :,
                                (w * k_cw + a) * 8 : (w * k_cw + a + nk) * 8,
                            ],
                            num_idxs=nk * 128,
                            num_idxs_reg=nidx_regs[nk],
                            elem_size=D,
                            single_packet=False,
                            queue_num=hh % 4,
                        )
                        add_dep_helper(
                            getattr(gi, "ins", gi),
                            getattr(lib_inst, "ins", lib_inst),
                            reason="mlp library before dma_gather",
                        )
                        a += nk
                    ps = psB.tile([128, D], f32, tag="agg", name=f"agg{w}")
                    for k in range(k_cw):
                        sc = (w * k_cw + k) * 128
                        nc.tensor.matmul(
                            ps[:],
                            s_t[:, sc : sc + 128],
                            g_t[:, k, :],
                            start=(k == 0),
                            stop=False,
                        )
                    # self-loop term: ps += I.T @ h'_w
                    nc.tensor.matmul(
                        ps[:],
                        ident_t[:],
                        selfsrc_t[:, w * D : (w + 1) * D],
                        start=False,
                        stop=True,
                    )
                    yield w, ps

            # ---- fused round 1 + L2 + proj head, pipelined per window ----
            for w, ps in agg_round(h1p_full, h1p_t):
                z_f = wpool.tile([128, D], f32, tag="zf")
                nc.scalar.activation(
                    z_f[:], ps[:], ACTF.Copy, scale=dis_t[:, w : w + 1]
                )
                z_b = wpool.tile([128, D], bf16, tag="zb")
                nc.scalar.activation(
                    z_b[:], ps[:], ACTF.Copy, scale=dis_t[:, w : w + 1]
                )
                nc.sync.dma_start(z_out[w * 128 : (w + 1) * 128, :], z_f[:])
                # transpose z chunk into zt columns; relu'd copy into rt
                psT = psA.tile([128, 4, 128], bf16, tag="tr", name=f"tr{w}")
                for kk in range(4):
                    nc.tensor.transpose(
                        psT[:, kk, :],
                        z_b[:, kk * 128 : (kk + 1) * 128],
                        ident_t[:],
                    )
                zt_w = wpool.tile([128, 4, 128], bf16, tag="ztw")
                rt_w = wpool.tile([128, 4, 128], bf16, tag="rtw")
                nc.vector.tensor_copy(zt_w[:], psT[:])
                nc.vector.tensor_scalar(
                    rt_w[:], psT[:], 0.0, None, op0=mybir.AluOpType.max
                )

                def zt_tile(kk, m):
                    return zt_w[:, kk, :]

                def rt_tile(kk, m):
                    return rt_w[:, kk, :]

                # L2 for this node chunk -> H2' shard
                ps2 = dense_layer(rt_tile, "w2t", w)
                scale_to(h2p_t[:, w * D : (w + 1) * D], ps2, w, "b2")
                nc.sync.dma_start(
                    h2p_sh[w * 128 : (w + 1) * 128, :],
                    h2p_t[:, w * D : (w + 1) * D],
                )
                # proj first layer for this chunk
                ps3 = dense_layer(zt_tile, "wp1t", w)
                p1_b = wpool.tile([128, D], bf16, tag="p1")
                if "bp1" in b_t:
                    tmp = wpool.tile([128, D], f32, tag="btmp")
                    nc.vector.tensor_tensor(
                        tmp[:], ps3[:], b_t["bp1"][:], op=mybir.AluOpType.add
                    )
                    nc.vector.tensor_scalar(
                        p1_b[:], tmp[:], 0.0, None, op0=mybir.AluOpType.max
                    )
                else:
                    nc.vector.tensor_scalar(
                        p1_b[:], ps3[:], 0.0, None, op0=mybir.AluOpType.max
                    )
                psT2 = psA.tile([128, 4, 128], bf16, tag="tr", name=f"tr2{w}")
                for kk in range(4):
                    nc.tensor.transpose(
                        psT2[:, kk, :],
                        p1_b[:, kk * 128 : (kk + 1) * 128],
                        ident_t[:],
                    )
                p1t_w = wpool.tile([128, 4, 128], bf16, tag="p1tw")
                nc.vector.tensor_copy(p1t_w[:], psT2[:])

                def p1t_tile(kk, m):
                    return p1t_w[:, kk, :]
                # proj second layer for this chunk
                ps4 = dense_layer(p1t_tile, "wp2t", w)
                pj_t = wpool.tile([128, D], f32, tag="pj")
                if "bp2" in b_t:
                    nc.vector.tensor_tensor(
                        pj_t[:], ps4[:], b_t["bp2"][:], op=mybir.AluOpType.add
                    )
                else:
                    nc.vector.tensor_copy(pj_t[:], ps4[:])
                nc.sync.dma_start(proj_out[w * 128 : (w + 1) * 128, :], pj_t[:])
                if w == 4:
                    nc.gpsimd.collective_compute(
                        "AllGather",
                        mybir.AluOpType.bypass,
                        ins=[h2p_sh[0:HALF, :]],
                        outs=[h2p_full[0 : NC * HALF, :]],
                        replica_groups=[core_ids],
                    )
            nc.gpsimd.collective_compute(
                "AllGather",
                mybir.AluOpType.bypass,
                ins=[h2p_sh[HALF:NPAD, :]],
                outs=[h2p_full[NC * HALF : 2 * NC * HALF, :]],
                replica_groups=[core_ids],
            )

            # ---- phase E: round 2 -> out ----
            for w, ps in agg_round(h2p_full, h2p_t):
                o_f = wpool.tile([128, D], f32, tag="of")
                nc.scalar.activation(
                    o_f[:], ps[:], ACTF.Copy, scale=dis_t[:, w : w + 1]
                )
                nc.sync.dma_start(out_out[w * 128 : (w + 1) * 128, :], o_f[:])

    lower_extended_insts(nc)
    return nc


def _host_prep(x, edge_index, W1, W2, Wp1, Wp2):
    src = np.asarray(edge_index[0], np.int64)
    dst = np.asarray(edge_index[1], np.int64)

    # degree includes self loops (norm definition), but self edges are
    # handled on-device via the identity matmul, not the gather.
    deg = (np.bincount(np.concatenate([dst, np.arange(N)]), minlength=N)).astype(
        np.float32
    )
    dis = (1.0 / np.sqrt(np.maximum(deg, 1.0))).astype(np.float32)

    owner = src // NPC
    local = src - owner * NPC
    HALF = NPAD // 2
    # AllGather halves land rank-major per half: [8*640 | 8*640]
    gather_row = np.where(
        local < HALF,
        owner * HALF + local,
        NC * HALF + owner * HALF + (local - HALF),
    )

    dst_core = dst // NPC
    dst_local = dst - dst_core * NPC  # [0, 1250)
    win = dst_local // 128
    dloc = dst_local - win * 128

    order = np.lexsort((dst_local, dst_core))
    g_sorted = gather_row[order]
    dc = dst_core[order]
    wn = win[order]
    dl = dloc[order]

    counts = np.zeros((NC, CH), np.int64)
    np.add.at(counts, (dc, wn), 1)
    flat_counts = counts.reshape(-1)
    starts = np.concatenate([[0], np.cumsum(flat_counts)])[:-1].reshape(NC, CH)

    # dedup per (core, window); k_cw from max unique count
    uniq = {}
    max_u = 0
    for c in range(NC):
        for w in range(CH):
            s0, n = starts[c, w], counts[c, w]
            rows = g_sorted[s0 : s0 + n]
            dd = dl[s0 : s0 + n]
            u, inv = np.unique(rows, return_inverse=True)
            uniq[(c, w)] = (u, inv, dd)
            max_u = max(max_u, len(u))
    k_cw = int(np.ceil(max_u / 128))
    wlen = k_cw * 128

    per_core = []
    for c in range(NC):
        idx_pad = np.zeros((CH, wlen), np.int64)
        s_tab = np.zeros((CH, wlen, 128), np.float32)
        for w in range(CH):
            u, inv, dd = uniq[(c, w)]
            idx_pad[w, : len(u)] = u
            np.add.at(s_tab[w], (inv, dd), 1.0)

        iw = idx_pad.reshape(CH, wlen // 16, 16).transpose(0, 2, 1)
        idx16 = np.tile(iw, (1, 8, 1)).transpose(1, 0, 2).reshape(128, -1)
        idx16 = np.ascontiguousarray(idx16, np.int16)

        # stab: [128, CH*k_cw*128]; col (w*k_cw+k)*128+d, part p = S[w, k*128+p, d]
        stab = (
            s_tab.reshape(CH, k_cw, 128, 128)
            .transpose(2, 0, 1, 3)
            .reshape(128, -1)
        )
        stab = np.ascontiguousarray(stab).astype(_BF16)

        xc = np.zeros((NPAD, D), np.float32)
        xc[:NPC] = x[c * NPC : (c + 1) * NPC]
        xt = xc.T.reshape(4, 128, NPAD).transpose(1, 0, 2).reshape(128, -1)
        xt = np.ascontiguousarray(xt).astype(_BF16)

        dis_c = np.zeros((NPAD,), np.float32)
        dis_c[:NPC] = dis[c * NPC : (c + 1) * NPC]
        dis_t = np.ascontiguousarray(dis_c.reshape(CH, 128).T, np.float32)

        per_core.append(
            {"xt": xt, "idx16": idx16, "stab": stab, "dis": dis_t}
        )

    def wtile(W):
        wt = (
            np.asarray(W, np.float32)
            .reshape(4, 128, D)
            .transpose(1, 0, 2)
            .reshape(128, -1)
        )
        return np.ascontiguousarray(wt).astype(_BF16)

    shared = {
        "w1t": wtile(W1),
        "w2t": wtile(W2),
        "wp1t": wtile(Wp1),
        "wp2t": wtile(Wp2),
        "ident": np.eye(128, dtype=np.float32).astype(_BF16),
    }
    return k_cw, per_core, shared


def run(inputs, trace=False, **run_kwargs):
    """Build + run; returns ((out, z, proj), BassKernelResults)."""
    _install_wait_split()
    from concourse.bass_utils import run_bass_kernel_spmd

    x = np.asarray(inputs["x"], np.float32)
    b1, b2 = inputs["b1"], inputs["b2"]
    bp1, bp2 = inputs["bp1"], inputs["bp2"]
    k_cw, per_core, shared = _host_prep(
        x, inputs["edge_index"], inputs["W1"], inputs["W2"], inputs["Wp1"],
        inputs["Wp2"],
    )

    has_b = {
        "b1": bool(np.any(np.asarray(b1))),
        "b2": bool(np.any(np.asarray(b2))),
        "bp1": bool(np.any(np.asarray(bp1))),
        "bp2": bool(np.any(np.asarray(bp2))),
    }
    nc = _build_program(
        k_cw, has_b["b1"], has_b["b2"], has_b["bp1"], has_b["bp2"]
    )

    in_maps = []
    for c in range(NC):
        m = dict(per_core[c])
        m.update(shared)
        for nm, b in (("b1", b1), ("b2", b2), ("bp1", bp1), ("bp2", bp2)):
            if has_b[nm]:
                m[nm] = np.ascontiguousarray(
                    np.tile(np.asarray(b, np.float32)[None, :], (128, 1))
                )
        in_maps.append(m)

    res = run_bass_kernel_spmd(
        nc, in_maps, core_ids=list(range(NC)), trace=trace, **run_kwargs
    )

    out = np.empty((N, D), np.float32)
    z = np.empty((N, D), np.float32)
    proj = np.empty((N, D), np.float32)
    for c in range(NC):
        r = res.results[c]
        out[c * NPC : (c + 1) * NPC] = r["agg"][:NPC]
        z[c * NPC : (c + 1) * NPC] = r["z"][:NPC]
        proj[c * NPC : (c + 1) * NPC] = r["proj"][:NPC]
    return (out, z, proj), res


def kernel(x, edge_index, W1, b1, W2, b2, Wp1, bp1, Wp2, bp2):
    outs, _ = run(
        {
            "x": x, "edge_index": edge_index, "W1": W1, "b1": b1,
            "W2": W2, "b2": b2, "Wp1": Wp1, "bp1": bp1,
            "Wp2": Wp2, "bp2": bp2,
        }
    )
    return outs

